# revision 1
# baseline (speedup 1.0000x reference)
"""Causal GQA attention (S=2048, B=2, HQ=32, HKV=8, D=128) on 8 trn2 cores.

Sharding: the 16 (batch, kv-head) pairs are split 2 per core (data+head
parallel). Each pair carries group=4 query heads -> 8 attention heads/core.

Device kernel computes, per head, S^T = (Q K^T)^T in PSUM chunk-by-chunk
(so the softmax free axis never needs an on-chip transpose), exponentiates
on ACT into SBUF (P^T), applies the causal triangular mask only on the
128x128 diagonal block, then accumulates out^T = V^T-style matmuls with V
stationary and the softmax denominators with a ones-column matmul. All
matmul operands are viewed as float32r (full-rate fp32 on the PE array for
moving dim >= 256).

Host side only re-lays-out data: Q/K are fed pre-transposed [d, s], V as
[k_local, ktile, d], and the returned out^T [d, s] is transposed back.
"""

import numpy as np

import concourse.bass as bass
import concourse.mybir as mybir
import concourse.tile as tile
from concourse import bacc, bass_utils
from concourse.masks import make_upper_triangular

S, B, HQ, HKV, D = 2048, 2, 32, 8, 128
G = HQ // HKV                      # 4 query heads per kv head
NCORES = 8
NPAIRS = B * HKV                   # 16 (batch, kv-head) pairs
PAIRS_PER_CORE = NPAIRS // NCORES  # 2
HEADS_PER_CORE = PAIRS_PER_CORE * G  # 8
SCALE = 1.0 / float(np.sqrt(D))
QC = 512                           # q-chunk (PSUM bank) width
NQC = S // QC                      # 4
KT = 128                           # k-tile (partition) width
NKT = S // KT                      # 16

F32 = mybir.dt.float32
F32R = mybir.dt.float32r
BF16 = mybir.dt.bfloat16


def emit_core_program(tc, qt, kt, v, recd, ot):
    """Emit the per-core program.

    qt: [HEADS_PER_CORE, D, S] f32r   Q^T per head ([d, q])
    kt: [PAIRS_PER_CORE, D, S] f32r   K^T per pair ([d, k])
    v:  [PAIRS_PER_CORE, 128, NKT*D] f32  V per pair ([k_local, kt, d])
    recd: [HEADS_PER_CORE, NQC, QC] f32 DRAM scratch for 1/sum rows
    ot: [HEADS_PER_CORE, D, S] f32   out^T per head ([d, q])

    QK^T runs in float32r (full-rate fp32); the P*V side runs in bf16
    (P in [0, e^~5], V order-1: bf16 keeps ~4e-3 relative accuracy and the
    softmax normalization cancels much of the P rounding).
    """
    from contextlib import ExitStack

    nc = tc.nc
    with ExitStack() as ctx:
        _emit_core_program(ctx, tc, nc, qt, kt, v, recd, ot)


def _emit_core_program(ctx, tc, nc, qt, kt, v, recd, ot):
    singles = ctx.enter_context(tc.tile_pool(name="singles", bufs=1))
    kv_pool = ctx.enter_context(tc.tile_pool(name="kv", bufs=2))
    q_pool = ctx.enter_context(tc.tile_pool(name="q", bufs=2))
    pt_pool = ctx.enter_context(tc.tile_pool(name="pt", bufs=3))
    ob_pool = ctx.enter_context(tc.tile_pool(name="ob", bufs=3))
    nrm_pool = ctx.enter_context(tc.tile_pool(name="nrm", bufs=3))
    ps_s = ctx.enter_context(tc.tile_pool(name="ps_s", bufs=1, space="PSUM"))
    ps_o = ctx.enter_context(tc.tile_pool(name="ps_o", bufs=5, space="PSUM"))
    ps_sum = ctx.enter_context(tc.tile_pool(name="ps_sum", bufs=1, space="PSUM"))

    # Constants
    # tri[k, q] = 1.0 where q >= k (allowed), 0.0 where q < k. Multiplied
    # into the P^T diagonal block after exp (bf16, off the QK->exp path).
    trif = singles.tile([128, 128], F32)
    make_upper_triangular(nc, trif[:], val=1.0, diag=True)
    tri = singles.tile([128, 128], BF16)
    nc.scalar.copy(out=tri[:], in_=trif[:])
    onesc = singles.tile([128, 1], BF16)   # ones column (sum-over-k lhsT)
    nc.vector.memset(onesc[:], 1.0)

    for pair in range(PAIRS_PER_CORE):
        kt_sb = kv_pool.tile([D, S], F32R, tag="kt")
        nc.sync.dma_start(out=kt_sb[:], in_=kt[pair])
        v_sb = kv_pool.tile([128, NKT * D], BF16, tag="v")
        nc.gpsimd.dma_start(out=v_sb[:], in_=v[pair])  # casting DMA f32->bf16

        for g in range(G):
            head = pair * G + g
            q_sb = q_pool.tile([D, S], F32R)
            nc.sync.dma_start(out=q_sb[:], in_=qt[head])

            s_ps = ps_s.tile([128, 2 * QC], F32)    # 2 banks of S^T staging
            # out^T accumulators: one PSUM bank per q-chunk, rotating through
            # 5 banks so the next head's chunk never WARs on this head's
            # in-flight normalization
            o_tiles = [ps_o.tile([128, QC], F32, tag="o", name=f"o_{head}_{c}")
                       for c in range(NQC)]
            sum_ps = ps_sum.tile([128, QC], F32)    # 1 bank: chunk c at row 32c

            norm_state = {}

            def norm_stage_a(c):
                # sums row PSUM -> SBUF (ACT), then DMA-reshape to [128, 4]
                # so the reciprocal runs 128 lanes wide
                row = slice(32 * c, 32 * c + 1)
                sr = nrm_pool.tile([128, QC], F32, tag="sumrow")
                nc.scalar.copy(out=sr[row, :], in_=sum_ps[row, :])
                srec = nrm_pool.tile([128, NQC], F32, tag="srec")
                nc.sync.dma_start(out=srec[:], in_=sr[row, :])
                norm_state[c] = srec

            def norm_stage_b(c):
                srec = norm_state[c]
                srec2 = nrm_pool.tile([128, NQC], F32, tag="srec2")
                nc.vector.reciprocal(out=srec2[:], in_=srec[:])
                nc.sync.dma_start(out=recd[head, c], in_=srec2[:])
                bcs = nrm_pool.tile([128, QC], F32, tag="bc")
                nc.sync.dma_start(
                    out=bcs[:], in_=recd[head, c].partition_broadcast(128))
                norm_state[c] = bcs

            def norm_stage_c(c):
                bcs = norm_state.pop(c)
                osb = ob_pool.tile([128, QC], F32)
                nc.vector.tensor_mul(osb[:], o_tiles[c][:], bcs[:])
                nc.sync.dma_start(
                    out=ot[head][:, QC * c:QC * (c + 1)], in_=osb[:])

            for kti in range(NKT):
                w = KT * kti          # first allowed q for this k-tile
                c0 = w // QC          # first overlapping q-chunk
                p_kt = pt_pool.tile([128, S], BF16)  # P^T rows for this k-tile

                def s_slice(c):
                    off = max(0, w - QC * c)
                    base = QC * ((c - c0) % 2)
                    return off, s_ps[:, base + off:base + QC]

                def av_ones(c):
                    off = max(0, w - QC * c)
                    rhs = p_kt[:, QC * c + off:QC * (c + 1)]
                    first = kti == 0
                    last = kti == 4 * c + 3
                    nc.tensor.matmul(
                        out=o_tiles[c][:, off:QC],
                        lhsT=v_sb[:, D * kti:D * (kti + 1)],
                        rhs=rhs, start=first, stop=last,
                    )
                    nc.tensor.matmul(
                        out=sum_ps[32 * c:32 * c + 1, off:QC],
                        lhsT=onesc[:],
                        rhs=rhs, start=first, stop=last,
                        tile_position=(0, 32 * c),
                    )

                # interleave QK -> exp -> (prev chunk AV) so PE always has a
                # runnable matmul while ACT exponentiates
                prev = None
                for c in range(c0, NQC):
                    off, s_ap = s_slice(c)
                    nc.tensor.matmul(
                        out=s_ap,
                        lhsT=kt_sb[:, w:w + KT],
                        rhs=q_sb[:, QC * c + off:QC * (c + 1)],
                        start=True, stop=True,
                    )
                    nc.scalar.activation(
                        p_kt[:, QC * c + off:QC * (c + 1)], s_ap,
                        mybir.ActivationFunctionType.Exp, scale=SCALE)
                    if c == c0:
                        # causal mask: zero q < k on the diagonal block
                        nc.vector.tensor_mul(
                            p_kt[:, w:w + KT], p_kt[:, w:w + KT], tri[:])
                    if prev is not None:
                        av_ones(prev)
                    prev = c
                av_ones(prev)

                # Normalization, software-pipelined across k-tile iterations
                # so the slow partition-broadcast DMA never blocks the DVE
                # stream: chunk c finishes accumulating at kti=4c+3 (stage A:
                # pull sums row + reshape), recip + broadcast issue at 4c+4
                # (stage B), multiply + store at 4c+5 (stage C).
                if kti >= 3 and (kti - 3) % 4 == 0:
                    norm_stage_a((kti - 3) // 4)
                if kti >= 4 and (kti - 4) % 4 == 0:
                    norm_stage_b((kti - 4) // 4)
                if kti >= 5 and (kti - 5) % 4 == 0:
                    norm_stage_c((kti - 5) // 4)

            # drain chunk 3 (finished at kti=15)
            norm_stage_b(3)
            norm_stage_c(3)


_CACHED_NC = None


def build_program():
    global _CACHED_NC
    if _CACHED_NC is not None:
        return _CACHED_NC
    nc = bacc.Bacc("TRN2", target_bir_lowering=False, debug=False,
                   num_devices=NCORES)
    qt = nc.dram_tensor("qt", [HEADS_PER_CORE, D, S], F32R,
                        kind="ExternalInput").ap()
    kt = nc.dram_tensor("kt", [PAIRS_PER_CORE, D, S], F32R,
                        kind="ExternalInput").ap()
    v = nc.dram_tensor("v", [PAIRS_PER_CORE, 128, NKT * D], F32,
                       kind="ExternalInput").ap()
    recd = nc.dram_tensor("recd", [HEADS_PER_CORE, NQC, QC], F32,
                          kind="Internal").ap()
    ot = nc.dram_tensor("ot", [HEADS_PER_CORE, D, S], F32,
                        kind="ExternalOutput").ap()
    with tile.TileContext(nc) as tc:
        emit_core_program(tc, qt, kt, v, recd, ot)
    nc.compile()
    _CACHED_NC = nc
    return nc


def shard_inputs(query, key, value):
    """Full inputs -> list of 8 per-core in_maps (host-side relayout only)."""
    query = np.asarray(query, dtype=np.float32)
    key = np.asarray(key, dtype=np.float32)
    value = np.asarray(value, dtype=np.float32)

    # Q: [S,B,HQ,D] -> [B*HKV, G, D, S]
    qtall = np.ascontiguousarray(
        query.reshape(S, B, HKV, G, D).transpose(1, 2, 3, 4, 0)
    ).reshape(NPAIRS, G, D, S)
    # K: [S,B,HKV,D] -> [B*HKV, D, S]
    ktall = np.ascontiguousarray(
        key.transpose(1, 2, 3, 0)).reshape(NPAIRS, D, S)
    # V: [S,B,HKV,D] -> [B*HKV, k_local=128, NKT*D]
    vall = np.ascontiguousarray(
        value.reshape(NKT, 128, B, HKV, D).transpose(2, 3, 1, 0, 4)
    ).reshape(NPAIRS, 128, NKT * D)

    in_maps = []
    for c in range(NCORES):
        p0 = PAIRS_PER_CORE * c
        p1 = p0 + PAIRS_PER_CORE
        in_maps.append({
            "qt": np.ascontiguousarray(qtall[p0:p1].reshape(HEADS_PER_CORE, D, S)),
            "kt": np.ascontiguousarray(ktall[p0:p1]),
            "v": np.ascontiguousarray(vall[p0:p1]),
        })
    return in_maps


def unshard_output(results):
    """8 per-core {'ot': [8, D, S]} -> full [S, B, HQ, D]."""
    ot = np.stack([r["ot"] for r in results])          # [8, 8, D, S]
    ot = ot.reshape(B, HKV, G, D, S)                   # pairs major -> b, hkv
    out = np.ascontiguousarray(ot.transpose(4, 0, 1, 2, 3))  # [S,B,HKV,G,D]
    return out.reshape(S, B, HQ, D)


def kernel(query, key, value, _trace=False, _return_bkr=False):
    nc = build_program()
    in_maps = shard_inputs(query, key, value)
    bkr = bass_utils.run_bass_kernel_spmd(
        nc, in_maps, core_ids=list(range(NCORES)), trace=_trace)
    out = unshard_output(bkr.results)
    if _return_bkr:
        return out, bkr
    return out


if __name__ == "__main__":
    q = np.random.randn(S, B, HQ, D).astype(np.float32)
    k = np.random.randn(S, B, HKV, D).astype(np.float32)
    vv = np.random.randn(S, B, HKV, D).astype(np.float32)
    o = kernel(q, k, vv)
    print("out", o.shape, o.dtype, float(np.abs(o).max()))



# revision 6
# speedup vs baseline: 2.4359x; 2.4359x over previous
"""Causal GQA attention (S=2048, B=2, HQ=32, HKV=8, D=128) on 8 trn2 cores.

Sharding: the 16 (batch, kv-head) pairs are split 2 per core (data+head
parallel). Each pair carries group=4 query heads -> 8 attention heads/core.

Per head the device kernel walks two 1024-wide q-chunks; for each chunk it
streams the causal k-tiles (128 wide): S^T = (Q K^T)^T lands in a 2-bank
PSUM staging tile (k on partitions, q on the free axis), one wide ACTIVATE
exponentiates it into SBUF (P^T, bf16), the 128x128 diagonal block is
masked by a triangular multiply, and V-stationary matmuls accumulate
out^T = V^T P^T into a persistent 2-bank PSUM accumulator. All matmul
operands are bf16 (1 col/cycle on the PE at full clock) and every matmul
is <=512 moving columns so no instruction straddles a PSUM bank.

Softmax denominators: k-tiles with kti%4==0 go straight to the PE as
ones-column matmuls into a shared PSUM sum bank (kti==0 opens the
accumulation with full chunk width); the other k-tiles are element-wise
accumulated on the DVE into a bf16 partial-sum tile, which a single pair
of ones-matmuls folds into the same PSUM rows at chunk end. This keeps
both PE and DVE under the ScalarE exp floor, which is the roofline here
(1 elem/lane/cycle @ 1.2 GHz over ~17.4M causal logits/core).

Chunk tails are software-pipelined: the accumulator is evacuated to SBUF
(bf16) immediately so the next chunk's matmuls can reuse the PSUM bank,
then sums -> SBUF -> DMA-reshape [128,8] -> reciprocal -> DRAM ->
partition-broadcast -> multiply -> store advances one stage per k-tile
iteration, several chunks in flight.

Host side only re-lays-out data (and casts to bf16): Q/K as [d, s], V as
[k_local, ktile*d]; the returned out^T [d, s] is transposed/cast back.
"""

import numpy as np
import ml_dtypes

import concourse.bass as bass
import concourse.mybir as mybir
import concourse.tile as tile
from concourse import bacc, bass_utils
from concourse.masks import make_upper_triangular

S, B, HQ, HKV, D = 2048, 2, 32, 8, 128
G = HQ // HKV                      # 4 query heads per kv head
NCORES = 8
NPAIRS = B * HKV                   # 16 (batch, kv-head) pairs
PAIRS_PER_CORE = NPAIRS // NCORES  # 2
HEADS_PER_CORE = PAIRS_PER_CORE * G  # 8
SCALE = 1.0 / float(np.sqrt(D))
CH = 1024                          # q-chunk width (2 PSUM banks)
NCH = S // CH                      # 2
KT = 128                           # k-tile (partition) width
NKT = S // KT                      # 16

F32 = mybir.dt.float32
BF16 = mybir.dt.bfloat16
NP_BF16 = ml_dtypes.bfloat16


def _segs(off):
    """Split chunk cols [off, CH) into <=512 pieces that don't straddle
    the 512 boundary (one PSUM bank per matmul)."""
    if off < 512:
        return [(off, 512), (512, CH)]
    return [(off, CH)]


def _sum_rc(row_base, s0, s1):
    """Map chunk cols [s0, s1) to (row, col range) in the 512-wide sum
    bank: lo half at row_base, hi half at row_base+32."""
    if s0 < 512:
        return row_base, s0, s1
    return row_base + 32, s0 - 512, s1 - 512


def emit_core_program(tc, qt, kt, v, recd, ot):
    from contextlib import ExitStack

    nc = tc.nc
    with ExitStack() as ctx:
        _emit_core_program(ctx, tc, nc, qt, kt, v, recd, ot)


def _emit_core_program(ctx, tc, nc, qt, kt, v, recd, ot):
    singles = ctx.enter_context(tc.tile_pool(name="singles", bufs=1))
    kv_pool = ctx.enter_context(tc.tile_pool(name="kv", bufs=2))
    q_pool = ctx.enter_context(tc.tile_pool(name="q", bufs=2))
    pt_pool = ctx.enter_context(tc.tile_pool(name="pt", bufs=3))
    sacc_pool = ctx.enter_context(tc.tile_pool(name="sacc", bufs=2))
    osb_pool = ctx.enter_context(tc.tile_pool(name="osb", bufs=3))
    bcs_pool = ctx.enter_context(tc.tile_pool(name="bcs", bufs=3))
    srow_pool = ctx.enter_context(tc.tile_pool(name="srow", bufs=3))
    srec_pool = ctx.enter_context(tc.tile_pool(name="srec", bufs=3))
    st_pool = ctx.enter_context(tc.tile_pool(name="st", bufs=2, space="PSUM"))
    oa_pool = ctx.enter_context(tc.tile_pool(name="oa", bufs=1, space="PSUM"))
    ps_sum = ctx.enter_context(tc.tile_pool(name="ps_sum", bufs=1, space="PSUM"))

    # Constants: tri[k, q] = 1.0 where q >= k (allowed), 0.0 where q < k.
    trif = singles.tile([128, 128], F32)
    make_upper_triangular(nc, trif[:], val=1.0, diag=True)
    tri = singles.tile([128, 128], BF16)
    nc.scalar.copy(out=tri[:], in_=trif[:])
    onesc = singles.tile([128, 1], BF16)   # ones column (sum-over-k lhsT)
    nc.vector.memset(onesc[:], 1.0)

    # One sum bank for the whole program; rows 0/32 and 64/96 alternate by
    # global chunk parity (subtile deps keep the parities independent).
    sum_ps = ps_sum.tile([128, 512], F32)

    kv_tiles = {}

    def ensure_pair(pair):
        if pair in kv_tiles or pair >= PAIRS_PER_CORE:
            return
        kt_sb = kv_pool.tile([D, S], BF16, tag="kt", name=f"kt_{pair}")
        nc.sync.dma_start(out=kt_sb[:], in_=kt[pair])
        v_sb = kv_pool.tile([128, NKT * D], BF16, tag="v", name=f"v_{pair}")
        nc.sync.dma_start(out=v_sb[:], in_=v[pair])
        kv_tiles[pair] = (kt_sb, v_sb)

    q_tiles = {}

    def ensure_head(head):
        if head in q_tiles or head >= HEADS_PER_CORE:
            return
        q_sb = q_pool.tile([D, S], BF16, tag="q", name=f"q_{head}")
        nc.sync.dma_start(out=q_sb[:], in_=qt[head])
        q_tiles[head] = q_sb

    # Flat schedule: (head, chunk, kti)
    sched = []
    for head in range(HEADS_PER_CORE):
        for c in range(NCH):
            for kti in range(8 * c + 8):
                sched.append((head, c, kti))

    # Per-(head,chunk) live state filled in while emitting
    oacc = {}      # (head, c) -> psum accumulator tile
    saccs = {}     # (head, c) -> (tile, base_off)
    stages = {}    # sched index -> staging tile

    # Chunk-tail normalization pipeline, advanced one stage per iteration
    pending = []

    def advance_norm(drain=False):
        for ent in list(pending):
            head, c, st = ent["head"], ent["c"], ent["stage"]
            if st == 0:
                srec = srec_pool.tile([128, NCH * 4], F32, tag="srec",
                                      name=f"srec_{head}_{c}")
                nc.gpsimd.dma_start(out=srec[:], in_=ent["srow"][:])
                ent["srec"] = srec
            elif st == 1:
                srec2 = srec_pool.tile([128, NCH * 4], BF16, tag="srec2",
                                       name=f"srec2_{head}_{c}")
                with nc.allow_low_precision(reason="1/sum broadcast in bf16"):
                    nc.vector.reciprocal(out=srec2[:], in_=ent["srec"][:])
                nc.gpsimd.dma_start(out=recd[head, c], in_=srec2[:])
            elif st == 2:
                bcs = bcs_pool.tile([128, CH], BF16, tag="bcs", name=f"bcs_{head}_{c}")
                nc.gpsimd.dma_start(
                    out=bcs[:], in_=recd[head, c].partition_broadcast(128))
                ent["bcs"] = bcs
            elif st == 3:
                osb2 = osb_pool.tile([128, CH], BF16, tag="osb2",
                                     name=f"osb2_{head}_{c}")
                nc.vector.tensor_mul(osb2[:], ent["osb"][:], ent["bcs"][:])
                nc.sync.dma_start(
                    out=ot[head][:, CH * c:CH * (c + 1)], in_=osb2[:])
                pending.remove(ent)
            ent["stage"] = st + 1

    def emit_qk(i):
        head, c, kti = sched[i]
        if c == 0 and kti == 0:
            pair = head // G
            ensure_pair(pair + 1)
            ensure_head(head + 1)
        kt_sb, _ = kv_tiles[head // G]
        q_sb = q_tiles[head]
        off = max(0, 128 * kti - CH * c)
        stage = st_pool.tile([128, CH], F32, tag="stage", name=f"st_{i}")
        for (s0, s1) in _segs(off):
            nc.tensor.matmul(
                out=stage[:, s0:s1],
                lhsT=kt_sb[:, 128 * kti:128 * (kti + 1)],
                rhs=q_sb[:, CH * c + s0:CH * c + s1],
                start=True, stop=True,
            )
        stages[i] = stage

    def emit_rest(i):
        head, c, kti = sched[i]
        _, v_sb = kv_tiles[head // G]
        off = max(0, 128 * kti - CH * c)
        last = kti == 8 * c + 7
        row_base = 64 * ((head * NCH + c) % 2)
        stage = stages.pop(i)

        # exp into SBUF (bf16); one wide ACTIVATE per k-tile
        p_kt = pt_pool.tile([128, CH], BF16, tag="pt", name=f"pt_{i}")
        nc.scalar.activation(
            p_kt[:, off:CH], stage[:, off:CH],
            mybir.ActivationFunctionType.Exp, scale=SCALE)

        # causal mask on the diagonal 128x128 block
        if 128 * kti >= CH * c:
            nc.vector.tensor_mul(
                p_kt[:, off:off + 128], p_kt[:, off:off + 128], tri[:])

        # out^T += V^T P^T
        if kti == 0:
            oacc[(head, c)] = oa_pool.tile(
                [128, CH], F32, tag="oacc", name=f"oa_{head}_{c}")
        oa = oacc[(head, c)]
        for (s0, s1) in _segs(off):
            nc.tensor.matmul(
                out=oa[:, s0:s1],
                lhsT=v_sb[:, D * kti:D * (kti + 1)],
                rhs=p_kt[:, s0:s1],
                start=(kti == 0), stop=last,
            )

        # denominators: kti%4==0 -> PE ones-matmul, else DVE accumulate
        if kti % 4 == 0:
            for (s0, s1) in _segs(off):
                r, c0, c1 = _sum_rc(row_base, s0, s1)
                nc.tensor.matmul(
                    out=sum_ps[r:r + 1, c0:c1],
                    lhsT=onesc[:],
                    rhs=p_kt[:, s0:s1],
                    start=(kti == 0), stop=False,
                    tile_position=(0, r),
                )
        else:
            key = (head, c)
            if key not in saccs:
                sacc = sacc_pool.tile([128, CH], BF16, tag="sacc", name=f"sacc_{head}_{c}")
                nc.vector.tensor_copy(sacc[:, off:CH], p_kt[:, off:CH])
                saccs[key] = (sacc, off)
            else:
                sacc, _ = saccs[key]
                nc.vector.tensor_add(
                    sacc[:, off:CH], sacc[:, off:CH], p_kt[:, off:CH])

        if last:
            # fold the DVE partial sums into the PSUM sum rows
            sacc, base = saccs.pop((head, c))
            for (s0, s1) in _segs(base):
                r, c0, c1 = _sum_rc(row_base, s0, s1)
                nc.tensor.matmul(
                    out=sum_ps[r:r + 1, c0:c1],
                    lhsT=onesc[:],
                    rhs=sacc[:, s0:s1],
                    start=False, stop=True,
                    tile_position=(0, r),
                )
            # evacuate the accumulator so the next chunk can reuse the bank
            oa = oacc.pop((head, c))
            osb = osb_pool.tile([128, CH], BF16, tag="osb",
                                name=f"osb_{head}_{c}")
            nc.vector.tensor_copy(osb[:], oa[:])
            # pull the two sum rows out of PSUM (single partition, 1024 wide)
            srow = srow_pool.tile([1, CH], F32, tag="srow", name=f"srow_{head}_{c}")
            nc.vector.tensor_copy(srow[0:1, 0:512], sum_ps[row_base:row_base + 1, :])
            nc.vector.tensor_copy(
                srow[0:1, 512:CH], sum_ps[row_base + 32:row_base + 33, :])
            pending.append(dict(head=head, c=c, stage=0, srow=srow, osb=osb))

    ensure_pair(0)
    ensure_head(0)
    emit_qk(0)
    for i in range(len(sched)):
        if i + 1 < len(sched):
            emit_qk(i + 1)
        emit_rest(i)
        advance_norm()
    while pending:
        advance_norm(drain=True)


_CACHED_NC = None


def build_program():
    global _CACHED_NC
    if _CACHED_NC is not None:
        return _CACHED_NC
    nc = bacc.Bacc("TRN2", target_bir_lowering=False, debug=False,
                   num_devices=NCORES)
    qt = nc.dram_tensor("qt", [HEADS_PER_CORE, D, S], BF16,
                        kind="ExternalInput").ap()
    kt = nc.dram_tensor("kt", [PAIRS_PER_CORE, D, S], BF16,
                        kind="ExternalInput").ap()
    v = nc.dram_tensor("v", [PAIRS_PER_CORE, 128, NKT * D], BF16,
                       kind="ExternalInput").ap()
    recd = nc.dram_tensor("recd", [HEADS_PER_CORE, NCH, CH], BF16,
                          kind="Internal").ap()
    ot = nc.dram_tensor("ot", [HEADS_PER_CORE, D, S], BF16,
                        kind="ExternalOutput").ap()
    with tile.TileContext(nc) as tc:
        emit_core_program(tc, qt, kt, v, recd, ot)
    nc.compile()
    _CACHED_NC = nc
    return nc


def shard_inputs(query, key, value):
    """Full inputs -> list of 8 per-core in_maps (host relayout + bf16)."""
    query = np.asarray(query, dtype=np.float32)
    key = np.asarray(key, dtype=np.float32)
    value = np.asarray(value, dtype=np.float32)

    # Q: [S,B,HQ,D] -> [B*HKV, G, D, S]
    qtall = np.ascontiguousarray(
        query.reshape(S, B, HKV, G, D).transpose(1, 2, 3, 4, 0)
    ).reshape(NPAIRS, G, D, S).astype(NP_BF16)
    # K: [S,B,HKV,D] -> [B*HKV, D, S]
    ktall = np.ascontiguousarray(
        key.transpose(1, 2, 3, 0)).reshape(NPAIRS, D, S).astype(NP_BF16)
    # V: [S,B,HKV,D] -> [B*HKV, k_local=128, NKT*D]
    vall = np.ascontiguousarray(
        value.reshape(NKT, 128, B, HKV, D).transpose(2, 3, 1, 0, 4)
    ).reshape(NPAIRS, 128, NKT * D).astype(NP_BF16)

    in_maps = []
    for c in range(NCORES):
        p0 = PAIRS_PER_CORE * c
        p1 = p0 + PAIRS_PER_CORE
        in_maps.append({
            "qt": np.ascontiguousarray(qtall[p0:p1].reshape(HEADS_PER_CORE, D, S)),
            "kt": np.ascontiguousarray(ktall[p0:p1]),
            "v": np.ascontiguousarray(vall[p0:p1]),
        })
    return in_maps


def unshard_output(results):
    """8 per-core {'ot': [8, D, S]} -> full [S, B, HQ, D]."""
    ot = np.stack([np.asarray(r["ot"], dtype=np.float32) for r in results])
    ot = ot.reshape(B, HKV, G, D, S)                   # pairs major -> b, hkv
    out = np.ascontiguousarray(ot.transpose(4, 0, 1, 2, 3))  # [S,B,HKV,G,D]
    return out.reshape(S, B, HQ, D)


def kernel(query, key, value, _trace=False, _return_bkr=False):
    nc = build_program()
    in_maps = shard_inputs(query, key, value)
    bkr = bass_utils.run_bass_kernel_spmd(
        nc, in_maps, core_ids=list(range(NCORES)), trace=_trace)
    out = unshard_output(bkr.results)
    if _return_bkr:
        return out, bkr
    return out


if __name__ == "__main__":
    q = np.random.randn(S, B, HQ, D).astype(np.float32)
    k = np.random.randn(S, B, HKV, D).astype(np.float32)
    vv = np.random.randn(S, B, HKV, D).astype(np.float32)
    o = kernel(q, k, vv)
    print("out", o.shape, o.dtype, float(np.abs(o).max()))
